# revision 35
# baseline (speedup 1.0000x reference)
"""CTC prefix scorer on Trainium2 — Bass/Tile kernel, SPMD over 8 NeuronCores.

Math (established against the reference; f32 baseline hit rel err ~6e-5):
the reference's 490-step lax.scan result is dead code, so per hypothesis h:

  log_psi[h, c] = log( sum_t w[t, h] * exp(x[b_h, t, c]) )

with w[t,h] = exp(rsum[t-1,h]) * [start <= t < xlen_b]      (normal labels)
         or = exp(r_prev[t-1,1,h]) * [...]                  (c == last_ids[h])
rsum = logaddexp(r_prev[:,0], r_prev[:,1]).  EOS col = rsum[xlen-1] (8
numbers per core — computed host-side from r_prev/xlens, like the weights),
BLANK/unscored cols = LOGZERO; all minus s_prev, folded host-side into `off`
(LOGZERO absorbs s_prev for unscored cols; the ~1e-9 rel slack is free).

Device program per core is just: stream x (f16, halves HBM traffic; abs err
<= |x|*2^-11), Exp -> bf16, 80 matmuls with host-built bf16 weights
(cols 0:8 wN, 32:40 wL, rest wm padding keeping every PSUM partition finite
under Ln), Ln each packed PSUM bank (two 500-col subtiles per bank at
64-row offsets -> full-partition ops), copy_predicated hit-merge, one
tensor_tensor add of `off`, store bf16.  Exp and Ln share the
`natural_log_exp_and_others` ACT table (forced by masking the other act
func sets) so they interleave with zero table loads.  All x DMAs are full
128 partitions — partial-partition DMAs round-robin over only 2 HW DMA
engines (observed), so the last chunk overlaps the previous one and its
duplicate weight rows are zeroed instead.
"""

import functools

import numpy as np
import ml_dtypes
from contextlib import ExitStack

import concourse.bass as bass
import concourse.tile as tile
from concourse import bacc, mybir
from concourse.bass_utils import run_bass_kernel_spmd
from concourse.tile_rust import add_dep_helper as _add_dep


def add_dep_helper(a, b, sync=True, reason=""):
    """a depends on b; unwrap BassInstruction -> mybir.Instruction."""
    _add_dep(getattr(a, "ins", a), getattr(b, "ins", b), sync=sync, reason=reason)

F32 = mybir.dt.float32
F16 = mybir.dt.float16
BF16 = mybir.dt.bfloat16
I32 = mybir.dt.int32
I16 = mybir.dt.int16
ACT = mybir.ActivationFunctionType
ALU = mybir.AluOpType

B, T, O = 8, 500, 10000
NH = 8                       # hypotheses per batch == per core
NCORES = 8
LOGZERO = -1e10
BLANK, EOS = 0, 2

NSUB = 500                   # output subtile width (PSUM bank: 500 f32 <= 2KB)
HALF = 5000                  # x load-chunk width (f16 rows: 10KB descriptors)
NBANK = O // (2 * NSUB)      # 10 banks, 2 subtiles each


def _patch_act_tables():
    """Restrict activation-table selection to `natural_log_exp_and_others`
    (full 400-bucket exp AND ln) so Exp/Ln interleave with zero
    ACT_TABLE_LOADs.  Other sets are emptied, not removed — the emitted
    act_func_set_id indexes the real act_info.json list."""
    import concourse.hw_specs as hs
    import concourse.bass_interp as bi

    target = "natural_log_exp_and_others"
    orig = hs.get_activation_tables
    if getattr(orig, "_ctc_patched", False):
        return

    @functools.cache
    def patched(arch):
        t = dict(orig(arch))
        if target in t:
            t = {k: (v if k == target else set()) for k, v in t.items()}
        return t

    patched._ctc_patched = True
    hs.get_activation_tables = patched
    bacc.get_activation_tables = patched
    bi.get_activation_tables = patched


def _chunks(start: int):
    """Full-128-row K-chunks [(t0, lo)] covering t in [start, T); the last
    chunk is shifted back to end exactly at T and `lo` marks the first row
    it owns (host zeroes weights for t < lo)."""
    out = []
    t0 = start
    while t0 + 128 < T:
        out.append((t0, t0))
        t0 += 128
    out.append((T - 128, t0))
    return out


def build_nc(start: int) -> bass.Bass:
    _patch_act_tables()
    CH = _chunks(start)
    NCH = len(CH)
    nc = bacc.Bacc(None)
    x_d = nc.declare_dram_parameter("x", [T, O], F16, isOutput=False)
    w_d = nc.declare_dram_parameter("wpk", [128, 64 * NCH], BF16, isOutput=False)
    pf_d = nc.declare_dram_parameter("pref", [128, 1 + NBANK], F32, isOutput=False)
    off_d = nc.declare_dram_parameter("off", [128, NBANK * NSUB], BF16,
                                      isOutput=False)
    out_d = nc.declare_dram_parameter("out", [NH, O], BF16, isOutput=True)

    with ExitStack() as ctx:
        tc = ctx.enter_context(tile.TileContext(nc))
        persist = ctx.enter_context(tc.tile_pool(name="persist", bufs=1))
        xrawp_f = ctx.enter_context(tc.tile_pool(name="xrawf", bufs=4))
        xrawp_h = ctx.enter_context(tc.tile_pool(name="xrawh", bufs=2))
        xrawp_q = ctx.enter_context(tc.tile_pool(name="xrawq", bufs=3))
        xtp_f = ctx.enter_context(tc.tile_pool(name="xtf", bufs=6))
        xtp_h = ctx.enter_context(tc.tile_pool(name="xth", bufs=2))
        xtp_q = ctx.enter_context(tc.tile_pool(name="xtq", bufs=3))

        def _sized(pools):
            f, h, q = pools
            return lambda w: f if w == HALF else (h if w == HALF // 2 else q)

        xraw_pool = _sized((xrawp_f, xrawp_h, xrawp_q))
        xt_pool = _sized((xtp_f, xtp_h, xtp_q))
        psum = ctx.enter_context(tc.tile_pool(name="ps", bufs=8, space="PSUM"))
        lgp = ctx.enter_context(tc.tile_pool(name="lg", bufs=6))

        # ---------------- DMA issues ----------------------------------------
        # sync q: small packed inputs first, then x chunks; scalar q: x
        # chunks, then off (first needed by the epilogue much later).
        wpk = persist.tile([128, 64 * NCH], BF16, tag="wpk")
        nc.gpsimd.dma_start(out=wpk[:], in_=w_d[:, :])
        pref = persist.tile([128, 1 + NBANK], F32, tag="pref")
        nc.gpsimd.dma_start(out=pref[:], in_=pf_d[:, :])
        eos_sb = pref[0:NH, 0:1]
        lidp = pref[:, 1:]

        # Three parallel DMA FIFOs: the two HWDGE queues plus the gpsimd
        # SWDGE queue (verified to stream 10KB-row transfers fine).  Each
        # queue's delivery cadence must stay ahead of the ACT engine's
        # chunk-consumption order; h1k1 is needed late so it rides the
        # slower SWDGE channel, thinning both HWDGE queues.
        # Per-segment queue map tuned so each queue's delivery order and
        # ~120GB/s concurrent-rate cadence beats the ACT consumption order:
        # q1 (sync), q10 (scalar), q0 (gpsimd SWDGE, late-needed chunks).
        ENG_MAP = {(0, 0): ["sync", "scalar", "sync"], (0, 1): ["scalar"],
                   (0, 2): ["sync"], (0, 3): ["gpsimd"],
                   (1, 0): ["sync"], (1, 1): ["gpsimd"],
                   (1, 2): ["scalar"],
                   (1, 3): ["sync", "scalar", "sync", "scalar", "sync"]}

        def issue_chunk(h, k, segs):
            c0 = h * HALF
            t0 = CH[k][0]
            engs = ENG_MAP[(h, k)]
            out = []
            for i, (sc0, w) in enumerate(segs):
                eng = getattr(nc, engs[i])
                xraw = xraw_pool(w).tile([128, w], F16, tag=f"xr{w}")
                eng.dma_start(out=xraw[:, :],
                              in_=x_d[t0:t0 + 128, c0 + sc0:c0 + sc0 + w])
                out.append((sc0, w, xraw))
            return out

        # first chunk split in two (faster pipeline start), last chunk of
        # half 1 split in five 2-subtile pieces (staggers the tail banks)
        SEG2 = [(0, 1000), (1000, 1500), (2500, 2500)]
        SEG5 = [(i * 1000, 1000) for i in range(5)]
        FULL = [(0, HALF)]
        segs0 = [SEG2] + [FULL] * (NCH - 1)
        segs1 = [FULL] * (NCH - 1) + [SEG5]
        xraws0 = [issue_chunk(0, k, segs0[k]) for k in range(NCH)]
        off_sb = persist.tile([128, NBANK * NSUB], BF16, tag="off")
        off_dma = nc.gpsimd.dma_start(out=off_sb[:], in_=off_d[:, :])
        iotac_i = persist.tile([128, NSUB], I32, tag="iotaci")
        nc.gpsimd.iota(iotac_i[:], pattern=[[1, NSUB]], base=0,
                       channel_multiplier=0)
        iotac = persist.tile([128, NSUB], F16, tag="iotac")
        nc.gpsimd.tensor_copy(out=iotac[:], in_=iotac_i[:])
        fin = persist.tile([NH, O], BF16, tag="fin")
        hitms = []
        for b in range(NBANK):
            hm = persist.tile([128, NSUB], I16, tag=f"hm{b}")
            nc.vector.tensor_scalar(out=hm[:], in0=iotac[:],
                                    scalar1=lidp[:, b:b + 1], scalar2=None,
                                    op0=ALU.is_equal)
            hitms.append(hm)

        # ---------------- pipeline ------------------------------------------
        def exp_seg(seg):
            sc0, w, xraw = seg
            xt = xt_pool(w).tile([128, w], BF16, tag=f"xt{w}")
            nc.scalar.activation(xt[:, :], xraw[:, :], ACT.Exp)
            return (sc0, w, xt)

        def mm(banks, xts_k, k, s):
            """One matmul: chunk k, subtile s, into its packed bank."""
            c0s = NSUB * s
            sc0, w, xt = next(t for t in xts_k if t[0] <= c0s < t[0] + t[1])
            nc.tensor.matmul(
                out=banks[s // 2][64 * (s % 2):64 * (s % 2) + 64, :],
                lhsT=wpk[:, 64 * k:64 * (k + 1)],
                rhs=xt[:, c0s - sc0:c0s - sc0 + NSUB],
                start=(k == 0), stop=(k == NCH - 1))

        def epilogue_bank(b, bank):
            lg = lgp.tile([128, NSUB], BF16, tag="lg")
            nc.scalar.activation(lg[:], bank[:], ACT.Ln)
            hitm = hitms[b]
            for j in range(2):
                cb = (2 * b + j) * NSUB
                nc.vector.copy_predicated(out=lg[64 * j:64 * j + NH, :],
                                          mask=hitm[64 * j + 32:64 * j + 40, :],
                                          data=lg[64 * j + 32:64 * j + 40, :])
                nc.vector.tensor_tensor(
                    out=fin[:, cb:cb + NSUB],
                    in0=lg[64 * j:64 * j + NH, :],
                    in1=off_sb[64 * j:64 * j + NH, b * NSUB:(b + 1) * NSUB],
                    op=ALU.add)
            if b == 0:
                # EOS col 2 lies in j=0's range; DVE is in-order after it
                nc.vector.tensor_tensor(out=fin[:, EOS:EOS + 1], in0=eos_sb,
                                        in1=off_sb[0:NH, EOS:EOS + 1],
                                        op=ALU.add)
            nc.sync.dma_start(out=out_d[:, 2 * b * NSUB:(2 * b + 2) * NSUB],
                              in_=fin[:, 2 * b * NSUB:(2 * b + 2) * NSUB])

        # half 0: exps, then k-outer matmul blocks (banks complete staggered
        # inside the final k block)
        # half-1 issues are interleaved between half-0 Exps so a DMA issue
        # waiting on an xraw slot never stalls the in-order ACT queue
        xts0 = []
        xraws1 = [None] * NCH
        issue1_after = {0: 0, 1: 1, 2: 2, 3: 2}   # h1 chunk -> after h0 exp k
        first_exp = None
        for k in range(NCH):
            row = []
            for seg in xraws0[k]:
                sc0, w, xraw = seg
                xt = xt_pool(w).tile([128, w], BF16, tag=f"xt{w}")
                ei = nc.scalar.activation(xt[:, :], xraw[:, :], ACT.Exp)
                if first_exp is None:
                    first_exp = ei
                row.append((sc0, w, xt))
            xts0.append(row)
            for k1, after in issue1_after.items():
                if after == k:
                    xraws1[k1] = issue_chunk(1, k1, segs1[k1])
        add_dep_helper(off_dma, first_exp, sync=True, reason="delay off")
        banks0 = [psum.tile([128, NSUB], F32, tag="bank", name=f"b0_{i}")
                  for i in range(5)]
        for k in range(NCH):
            for s in range(10):
                mm(banks0, xts0[k], k, s)

        # half 1: the three full exps interleave with half-0 epilogues on the
        # ACT queue; their matmul blocks follow.  The last chunk streams in
        # five pieces: exp piece -> its two k=3 matmuls -> that bank's
        # epilogue, so the tail pipeline is piecewise instead of monolithic.
        xts1_full = []
        epi0 = [(0, 1), (2, 3), (4,)]
        for k in range(NCH - 1):
            xts1_full.append([exp_seg(s) for s in xraws1[k]])
            for b in epi0[k]:
                epilogue_bank(b, banks0[b])
        banks1 = [psum.tile([128, NSUB], F32, tag="bank", name=f"b1_{i}")
                  for i in range(5)]
        for k in range(NCH - 1):
            for s in range(10):
                mm(banks1, xts1_full[k], k, s)
        for b in range(5):
            piece = exp_seg(xraws1[NCH - 1][b])
            for s in (2 * b, 2 * b + 1):
                mm(banks1, [piece], NCH - 1, s)
            epilogue_bank(5 + b, banks1[b])


    nc.compile()
    return nc


def make_in_maps(x, r_prev, s_prev, xlens, last_ids, scoring_ids, start):
    """Per-core input maps: core i owns batch i / hypotheses [8i, 8i+8)."""
    CH = _chunks(start)
    NCH = len(CH)
    in_maps = []
    r_prev = np.asarray(r_prev, np.float64)
    e1 = np.exp(r_prev[:, 1, :])                       # (T, n_bh)
    rsum = np.exp(r_prev[:, 0, :]) + e1
    for i in range(NCORES):
        hs = slice(i * NH, (i + 1) * NH)
        sids = np.ascontiguousarray(scoring_ids[hs]).astype(np.int64)  # (8,200)
        xlen = int(xlens[i])
        # off = -s_prev where scored, LOGZERO otherwise (absorbs -s_prev for
        # unscored: 1e10 dwarfs it).  BLANK forced LOGZERO; EOS forced
        # -s_prev (device adds the eos score there).  Packed to the lg
        # layout: row 64j+h, col b*NSUB+c <-> column (2b+j)*NSUB+c.
        off = np.full((NH, O), LOGZERO, np.float32)
        np.put_along_axis(off, sids, np.take_along_axis(-s_prev[hs], sids, 1), 1)
        off[:, EOS] = -s_prev[hs][:, EOS]
        off[:, BLANK] = LOGZERO
        off_pk = np.zeros((128, NBANK * NSUB), np.float32)
        for b in range(NBANK):
            for j in range(2):
                off_pk[64 * j:64 * j + NH, b * NSUB:(b + 1) * NSUB] = \
                    off[:, (2 * b + j) * NSUB:(2 * b + j + 1) * NSUB]
        # weights, chunk-packed: row p col 64k+m <-> w[t0_k+p, m];
        # w[t] = [wN(8) | wm(24) | wL(8) | wm(24)], wm = [lo_k<=t<xlen]
        # (lo_k excludes rows duplicated by the shifted last chunk)
        wpk = np.zeros((128, 64 * NCH), np.float32)
        for k, (t0, lo) in enumerate(CH):
            t = np.arange(t0, t0 + 128)
            wm = ((t >= lo) & (t < xlen)).astype(np.float64)   # (128,)
            wN = rsum[t - 1][:, hs] * wm[:, None]              # (128,8)
            wL = e1[t - 1][:, hs] * wm[:, None]
            wpk[:, 64 * k + 0:64 * k + 8] = wN
            wpk[:, 64 * k + 8:64 * k + 32] = wm[:, None]
            wpk[:, 64 * k + 32:64 * k + 40] = wL
            wpk[:, 64 * k + 40:64 * k + 64] = wm[:, None]
        # pref: col 0 rows 0:8 = eos = log(rsum[xlen-1]); cols 1: = lidp
        # (per bank b rows 32+h / 96+h: last_ids[h] - subtile colbase)
        # f16 holds integers exactly up to 2048; out-of-range lidp values
        # only need to stay outside [0, NSUB), so clip to -2047
        li = np.ascontiguousarray(last_ids[hs]).astype(np.int64)
        pref = np.full((128, 1 + NBANK), -2047, np.float32)
        pref[0:NH, 0] = np.log(rsum[xlen - 1][hs])
        for b in range(NBANK):
            for j in range(2):
                v = li - (2 * b + j) * NSUB
                pref[32 + 64 * j:40 + 64 * j, 1 + b] = np.clip(v, -2047, 2047)
        # hit masks, packed per bank: rows 64j+32+h of col block b flag the
        # column where last_ids[h] falls in subtile 2b+j
        hitm = np.zeros((128, NBANK * NSUB), np.int16)
        for b in range(NBANK):
            for j in range(2):
                for h in range(NH):
                    c = int(li[h]) - (2 * b + j) * NSUB
                    if 0 <= c < NSUB:
                        hitm[64 * j + 32 + h, b * NSUB + c] = 1
        in_maps.append({
            "x": np.ascontiguousarray(x[i]).astype(np.float16),
            "hitm": hitm,
            "wpk": wpk.astype(ml_dtypes.bfloat16),
            "pref": pref,
            "off": off_pk.astype(ml_dtypes.bfloat16),
        })
    return in_maps


_NC_CACHE: dict[int, bass.Bass] = {}


def kernel(x, r_prev, s_prev, xlens, last_ids, scoring_ids, output_length,
           _trace=False):
    x = np.asarray(x)
    r_prev = np.asarray(r_prev)
    s_prev = np.asarray(s_prev)
    xlens = np.asarray(xlens)
    last_ids = np.asarray(last_ids)
    scoring_ids = np.asarray(scoring_ids)
    start = max(int(output_length), 1)
    assert int(output_length) >= 1, "output_length==0 path not implemented"

    if start not in _NC_CACHE:
        _NC_CACHE[start] = build_nc(start)
    nc = _NC_CACHE[start]

    in_maps = make_in_maps(x, r_prev, s_prev, xlens, last_ids, scoring_ids,
                           start)
    res = run_bass_kernel_spmd(nc, in_maps, core_ids=list(range(NCORES)),
                               trace=_trace)
    out = np.concatenate(
        [np.asarray(res.results[i]["out"]).astype(np.float32)
         for i in range(NCORES)], axis=0)
    kernel.last_exec_time_ns = res.exec_time_ns
    kernel.last_results = res
    return out


# revision 36
# speedup vs baseline: 1.1883x; 1.1883x over previous
"""CTC prefix scorer on Trainium2 — Bass/Tile kernel, SPMD over 8 NeuronCores.

Math (established against the reference; f32 baseline hit rel err ~6e-5):
the reference's 490-step lax.scan result is dead code, so per hypothesis h:

  log_psi[h, c] = log( sum_t w[t, h] * exp(x[b_h, t, c]) )

with w[t,h] = exp(rsum[t-1,h]) * [start <= t < xlen_b]      (normal labels)
         or = exp(r_prev[t-1,1,h]) * [...]                  (c == last_ids[h])
rsum = logaddexp(r_prev[:,0], r_prev[:,1]).  EOS col = rsum[xlen-1] (8
numbers per core — computed host-side from r_prev/xlens, like the weights),
BLANK/unscored cols = LOGZERO; all minus s_prev, folded host-side into `off`
(LOGZERO absorbs s_prev for unscored cols; the ~1e-9 rel slack is free).

Device program per core is just: stream x (f16, halves HBM traffic; abs err
<= |x|*2^-11), Exp -> bf16, 80 matmuls with host-built bf16 weights
(cols 0:8 wN, 32:40 wL, rest wm padding keeping every PSUM partition finite
under Ln), Ln each packed PSUM bank (two 500-col subtiles per bank at
64-row offsets -> full-partition ops), copy_predicated hit-merge, one
tensor_tensor add of `off`, store bf16.  Exp and Ln share the
`natural_log_exp_and_others` ACT table (forced by masking the other act
func sets) so they interleave with zero table loads.  All x DMAs are full
128 partitions — partial-partition DMAs round-robin over only 2 HW DMA
engines (observed), so the last chunk overlaps the previous one and its
duplicate weight rows are zeroed instead.
"""

import functools

import numpy as np
import ml_dtypes
from contextlib import ExitStack

import concourse.bass as bass
import concourse.tile as tile
from concourse import bacc, mybir
from concourse.bass_utils import run_bass_kernel_spmd
from concourse.tile_rust import add_dep_helper as _add_dep


def add_dep_helper(a, b, sync=True, reason=""):
    """a depends on b; unwrap BassInstruction -> mybir.Instruction."""
    _add_dep(getattr(a, "ins", a), getattr(b, "ins", b), sync=sync, reason=reason)

F32 = mybir.dt.float32
F16 = mybir.dt.float16
BF16 = mybir.dt.bfloat16
I32 = mybir.dt.int32
I16 = mybir.dt.int16
ACT = mybir.ActivationFunctionType
ALU = mybir.AluOpType

B, T, O = 8, 500, 10000
NH = 8                       # hypotheses per batch == per core
NCORES = 8
LOGZERO = -1e10
BLANK, EOS = 0, 2

NSUB = 500                   # output subtile width (PSUM bank: 500 f32 <= 2KB)
HALF = 5000                  # x load-chunk width (f16 rows: 10KB descriptors)
NBANK = O // (2 * NSUB)      # 10 banks, 2 subtiles each


def _patch_act_tables():
    """Restrict activation-table selection to `natural_log_exp_and_others`
    (full 400-bucket exp AND ln) so Exp/Ln interleave with zero
    ACT_TABLE_LOADs.  Other sets are emptied, not removed — the emitted
    act_func_set_id indexes the real act_info.json list."""
    import concourse.hw_specs as hs
    import concourse.bass_interp as bi

    target = "natural_log_exp_and_others"
    orig = hs.get_activation_tables
    if getattr(orig, "_ctc_patched", False):
        return

    @functools.cache
    def patched(arch):
        t = dict(orig(arch))
        if target in t:
            t = {k: (v if k == target else set()) for k, v in t.items()}
        return t

    patched._ctc_patched = True
    hs.get_activation_tables = patched
    bacc.get_activation_tables = patched
    bi.get_activation_tables = patched


def _chunks(start: int):
    """Full-128-row K-chunks [(t0, lo)] covering t in [start, T); the last
    chunk is shifted back to end exactly at T and `lo` marks the first row
    it owns (host zeroes weights for t < lo)."""
    out = []
    t0 = start
    while t0 + 128 < T:
        out.append((t0, t0))
        t0 += 128
    out.append((T - 128, t0))
    return out


def build_nc(start: int) -> bass.Bass:
    _patch_act_tables()
    CH = _chunks(start)
    NCH = len(CH)
    nc = bacc.Bacc(None)
    x_d = nc.declare_dram_parameter("x", [T, O], F16, isOutput=False)
    w_d = nc.declare_dram_parameter("wpk", [128, 64 * NCH], BF16, isOutput=False)
    pf_d = nc.declare_dram_parameter("pref", [128, 1 + NBANK], F32, isOutput=False)
    off_d = nc.declare_dram_parameter("off", [128, NBANK * NSUB], BF16,
                                      isOutput=False)
    out_d = nc.declare_dram_parameter("out", [NH, O], BF16, isOutput=True)

    with ExitStack() as ctx:
        tc = ctx.enter_context(tile.TileContext(nc))
        persist = ctx.enter_context(tc.tile_pool(name="persist", bufs=1))
        xrawp_f = ctx.enter_context(tc.tile_pool(name="xrawf", bufs=4))
        xrawp_h = ctx.enter_context(tc.tile_pool(name="xrawh", bufs=2))
        xrawp_q = ctx.enter_context(tc.tile_pool(name="xrawq", bufs=5))
        xtp_f = ctx.enter_context(tc.tile_pool(name="xtf", bufs=6))
        xtp_h = ctx.enter_context(tc.tile_pool(name="xth", bufs=2))
        xtp_q = ctx.enter_context(tc.tile_pool(name="xtq", bufs=5))

        def _sized(pools):
            f, h, q = pools
            return lambda w: f if w == HALF else (h if w == HALF // 2 else q)

        xraw_pool = _sized((xrawp_f, xrawp_h, xrawp_q))
        xt_pool = _sized((xtp_f, xtp_h, xtp_q))
        psum = ctx.enter_context(tc.tile_pool(name="ps", bufs=8, space="PSUM"))
        lgp = ctx.enter_context(tc.tile_pool(name="lg", bufs=6))

        # ---------------- DMA issues ----------------------------------------
        # sync q: small packed inputs first, then x chunks; scalar q: x
        # chunks, then off (first needed by the epilogue much later).
        wpk = persist.tile([128, 64 * NCH], BF16, tag="wpk")
        nc.gpsimd.dma_start(out=wpk[:], in_=w_d[:, :])
        pref = persist.tile([128, 1 + NBANK], F32, tag="pref")
        nc.gpsimd.dma_start(out=pref[:], in_=pf_d[:, :])
        eos_sb = pref[0:NH, 0:1]
        lidp = pref[:, 1:]

        # Three parallel DMA FIFOs: the two HWDGE queues plus the gpsimd
        # SWDGE queue (verified to stream 10KB-row transfers fine).  Each
        # queue's delivery cadence must stay ahead of the ACT engine's
        # chunk-consumption order; h1k1 is needed late so it rides the
        # slower SWDGE channel, thinning both HWDGE queues.
        ENG_MAP = {(0, 0): "sync", (0, 1): "scalar", (0, 2): "scalar",
                   (0, 3): "sync", (1, 0): "scalar", (1, 1): "gpsimd",
                   (1, 2): "scalar", (1, 3): "sync"}

        def issue_chunk(h, k, segs):
            c0 = h * HALF
            t0 = CH[k][0]
            eng = getattr(nc, ENG_MAP[(h, k)])
            out = []
            for sc0, w in segs:
                xraw = xraw_pool(w).tile([128, w], F16, tag=f"xr{w}")
                eng.dma_start(out=xraw[:, :],
                              in_=x_d[t0:t0 + 128, c0 + sc0:c0 + sc0 + w])
                out.append((sc0, w, xraw))
            return out

        # first chunk split in two (faster pipeline start), last chunk of
        # half 1 split in five 2-subtile pieces (staggers the tail banks)
        SEG2 = [(0, HALF // 2), (HALF // 2, HALF // 2)]
        SEG5 = [(i * 1000, 1000) for i in range(5)]
        FULL = [(0, HALF)]
        segs0 = [SEG2] + [FULL] * (NCH - 1)
        segs1 = [FULL] * (NCH - 1) + [SEG5]
        xraws0 = [issue_chunk(0, k, segs0[k]) for k in range(NCH)]
        off_sb = persist.tile([128, NBANK * NSUB], BF16, tag="off")
        off_dma = nc.gpsimd.dma_start(out=off_sb[:], in_=off_d[:, :])
        iotac_i = persist.tile([128, NSUB], I32, tag="iotaci")
        nc.gpsimd.iota(iotac_i[:], pattern=[[1, NSUB]], base=0,
                       channel_multiplier=0)
        iotac = persist.tile([128, NSUB], F16, tag="iotac")
        nc.gpsimd.tensor_copy(out=iotac[:], in_=iotac_i[:])
        fin = persist.tile([NH, O], BF16, tag="fin")
        hitms = []
        for b in range(NBANK):
            hm = persist.tile([128, NSUB], I16, tag=f"hm{b}")
            nc.vector.tensor_scalar(out=hm[:], in0=iotac[:],
                                    scalar1=lidp[:, b:b + 1], scalar2=None,
                                    op0=ALU.is_equal)
            hitms.append(hm)

        # ---------------- pipeline ------------------------------------------
        def exp_seg(seg):
            sc0, w, xraw = seg
            xt = xt_pool(w).tile([128, w], BF16, tag=f"xt{w}")
            nc.scalar.activation(xt[:, :], xraw[:, :], ACT.Exp)
            return (sc0, w, xt)

        def mm(banks, xts_k, k, s):
            """One matmul: chunk k, subtile s, into its packed bank."""
            c0s = NSUB * s
            sc0, w, xt = next(t for t in xts_k if t[0] <= c0s < t[0] + t[1])
            nc.tensor.matmul(
                out=banks[s // 2][64 * (s % 2):64 * (s % 2) + 64, :],
                lhsT=wpk[:, 64 * k:64 * (k + 1)],
                rhs=xt[:, c0s - sc0:c0s - sc0 + NSUB],
                start=(k == 0), stop=(k == NCH - 1))

        def epilogue_bank(b, bank):
            lg = lgp.tile([128, NSUB], BF16, tag="lg")
            nc.scalar.activation(lg[:], bank[:], ACT.Ln)
            hitm = hitms[b]
            for j in range(2):
                cb = (2 * b + j) * NSUB
                nc.vector.copy_predicated(out=lg[64 * j:64 * j + NH, :],
                                          mask=hitm[64 * j + 32:64 * j + 40, :],
                                          data=lg[64 * j + 32:64 * j + 40, :])
                nc.vector.tensor_tensor(
                    out=fin[:, cb:cb + NSUB],
                    in0=lg[64 * j:64 * j + NH, :],
                    in1=off_sb[64 * j:64 * j + NH, b * NSUB:(b + 1) * NSUB],
                    op=ALU.add)
            if b == 0:
                # EOS col 2 lies in j=0's range; DVE is in-order after it
                nc.vector.tensor_tensor(out=fin[:, EOS:EOS + 1], in0=eos_sb,
                                        in1=off_sb[0:NH, EOS:EOS + 1],
                                        op=ALU.add)
            nc.sync.dma_start(out=out_d[:, 2 * b * NSUB:(2 * b + 2) * NSUB],
                              in_=fin[:, 2 * b * NSUB:(2 * b + 2) * NSUB])

        # half 0: exps, then k-outer matmul blocks (banks complete staggered
        # inside the final k block)
        # half-1 issues are interleaved between half-0 Exps so a DMA issue
        # waiting on an xraw slot never stalls the in-order ACT queue
        xts0 = []
        xraws1 = [None] * NCH
        issue1_after = {0: 0, 1: 1, 2: 2, 3: 2}   # h1 chunk -> after h0 exp k
        first_exp = None
        for k in range(NCH):
            row = []
            for seg in xraws0[k]:
                sc0, w, xraw = seg
                xt = xt_pool(w).tile([128, w], BF16, tag=f"xt{w}")
                ei = nc.scalar.activation(xt[:, :], xraw[:, :], ACT.Exp)
                if first_exp is None:
                    first_exp = ei
                row.append((sc0, w, xt))
            xts0.append(row)
            for k1, after in issue1_after.items():
                if after == k:
                    xraws1[k1] = issue_chunk(1, k1, segs1[k1])
        add_dep_helper(off_dma, first_exp, sync=True, reason="delay off")
        banks0 = [psum.tile([128, NSUB], F32, tag="bank", name=f"b0_{i}")
                  for i in range(5)]
        for k in range(NCH):
            for s in range(10):
                mm(banks0, xts0[k], k, s)

        # half 1: the three full exps interleave with half-0 epilogues on the
        # ACT queue; their matmul blocks follow.  The last chunk streams in
        # five pieces: exp piece -> its two k=3 matmuls -> that bank's
        # epilogue, so the tail pipeline is piecewise instead of monolithic.
        xts1_full = []
        epi0 = [(0, 1), (2, 3), (4,)]
        for k in range(NCH - 1):
            xts1_full.append([exp_seg(s) for s in xraws1[k]])
            for b in epi0[k]:
                epilogue_bank(b, banks0[b])
        banks1 = [psum.tile([128, NSUB], F32, tag="bank", name=f"b1_{i}")
                  for i in range(5)]
        for k in range(NCH - 1):
            for s in range(10):
                mm(banks1, xts1_full[k], k, s)
        for b in range(5):
            piece = exp_seg(xraws1[NCH - 1][b])
            for s in (2 * b, 2 * b + 1):
                mm(banks1, [piece], NCH - 1, s)
            epilogue_bank(5 + b, banks1[b])


    nc.compile()
    return nc


def make_in_maps(x, r_prev, s_prev, xlens, last_ids, scoring_ids, start):
    """Per-core input maps: core i owns batch i / hypotheses [8i, 8i+8)."""
    CH = _chunks(start)
    NCH = len(CH)
    in_maps = []
    r_prev = np.asarray(r_prev, np.float64)
    e1 = np.exp(r_prev[:, 1, :])                       # (T, n_bh)
    rsum = np.exp(r_prev[:, 0, :]) + e1
    for i in range(NCORES):
        hs = slice(i * NH, (i + 1) * NH)
        sids = np.ascontiguousarray(scoring_ids[hs]).astype(np.int64)  # (8,200)
        xlen = int(xlens[i])
        # off = -s_prev where scored, LOGZERO otherwise (absorbs -s_prev for
        # unscored: 1e10 dwarfs it).  BLANK forced LOGZERO; EOS forced
        # -s_prev (device adds the eos score there).  Packed to the lg
        # layout: row 64j+h, col b*NSUB+c <-> column (2b+j)*NSUB+c.
        off = np.full((NH, O), LOGZERO, np.float32)
        np.put_along_axis(off, sids, np.take_along_axis(-s_prev[hs], sids, 1), 1)
        off[:, EOS] = -s_prev[hs][:, EOS]
        off[:, BLANK] = LOGZERO
        off_pk = np.zeros((128, NBANK * NSUB), np.float32)
        for b in range(NBANK):
            for j in range(2):
                off_pk[64 * j:64 * j + NH, b * NSUB:(b + 1) * NSUB] = \
                    off[:, (2 * b + j) * NSUB:(2 * b + j + 1) * NSUB]
        # weights, chunk-packed: row p col 64k+m <-> w[t0_k+p, m];
        # w[t] = [wN(8) | wm(24) | wL(8) | wm(24)], wm = [lo_k<=t<xlen]
        # (lo_k excludes rows duplicated by the shifted last chunk)
        wpk = np.zeros((128, 64 * NCH), np.float32)
        for k, (t0, lo) in enumerate(CH):
            t = np.arange(t0, t0 + 128)
            wm = ((t >= lo) & (t < xlen)).astype(np.float64)   # (128,)
            wN = rsum[t - 1][:, hs] * wm[:, None]              # (128,8)
            wL = e1[t - 1][:, hs] * wm[:, None]
            wpk[:, 64 * k + 0:64 * k + 8] = wN
            wpk[:, 64 * k + 8:64 * k + 32] = wm[:, None]
            wpk[:, 64 * k + 32:64 * k + 40] = wL
            wpk[:, 64 * k + 40:64 * k + 64] = wm[:, None]
        # pref: col 0 rows 0:8 = eos = log(rsum[xlen-1]); cols 1: = lidp
        # (per bank b rows 32+h / 96+h: last_ids[h] - subtile colbase)
        # f16 holds integers exactly up to 2048; out-of-range lidp values
        # only need to stay outside [0, NSUB), so clip to -2047
        li = np.ascontiguousarray(last_ids[hs]).astype(np.int64)
        pref = np.full((128, 1 + NBANK), -2047, np.float32)
        pref[0:NH, 0] = np.log(rsum[xlen - 1][hs])
        for b in range(NBANK):
            for j in range(2):
                v = li - (2 * b + j) * NSUB
                pref[32 + 64 * j:40 + 64 * j, 1 + b] = np.clip(v, -2047, 2047)
        # hit masks, packed per bank: rows 64j+32+h of col block b flag the
        # column where last_ids[h] falls in subtile 2b+j
        hitm = np.zeros((128, NBANK * NSUB), np.int16)
        for b in range(NBANK):
            for j in range(2):
                for h in range(NH):
                    c = int(li[h]) - (2 * b + j) * NSUB
                    if 0 <= c < NSUB:
                        hitm[64 * j + 32 + h, b * NSUB + c] = 1
        in_maps.append({
            "x": np.ascontiguousarray(x[i]).astype(np.float16),
            "hitm": hitm,
            "wpk": wpk.astype(ml_dtypes.bfloat16),
            "pref": pref,
            "off": off_pk.astype(ml_dtypes.bfloat16),
        })
    return in_maps


_NC_CACHE: dict[int, bass.Bass] = {}


def kernel(x, r_prev, s_prev, xlens, last_ids, scoring_ids, output_length,
           _trace=False):
    x = np.asarray(x)
    r_prev = np.asarray(r_prev)
    s_prev = np.asarray(s_prev)
    xlens = np.asarray(xlens)
    last_ids = np.asarray(last_ids)
    scoring_ids = np.asarray(scoring_ids)
    start = max(int(output_length), 1)
    assert int(output_length) >= 1, "output_length==0 path not implemented"

    if start not in _NC_CACHE:
        _NC_CACHE[start] = build_nc(start)
    nc = _NC_CACHE[start]

    in_maps = make_in_maps(x, r_prev, s_prev, xlens, last_ids, scoring_ids,
                           start)
    res = run_bass_kernel_spmd(nc, in_maps, core_ids=list(range(NCORES)),
                               trace=_trace)
    out = np.concatenate(
        [np.asarray(res.results[i]["out"]).astype(np.float32)
         for i in range(NCORES)], axis=0)
    kernel.last_exec_time_ns = res.exec_time_ns
    kernel.last_results = res
    return out
